# revision 21
# baseline (speedup 1.0000x reference)
"""Trainium2 Bass kernel for causal self-attention (GQA + RoPE).

Problem: B=2, T=2048, n_embd=4096, HQ=32 q-heads, HKV=8 kv-heads, HD=128.
  q = rope(x @ wq), k = rope(x @ wk), v = x @ wv
  y = causal_softmax(q k^T / sqrt(HD)) v @ wproj

Sharding (8 cores): core = (b, g), b in {0,1} batch, g in {0..3} head-group.
Each core handles 8 q-heads / 2 kv-heads of one batch sample:
  - wq/wk/wv column-sharded, wproj row-sharded (tensor parallel over heads)
  - final reduce (sum of 4 partial y per batch) done on host in fp32.

Per-core device program (all matmuls bf16, fp32 accumulation):
  A) projections: Q^T,K^T per head [d=128 part, t free] with fused RoPE;
     V^T then PE-transposed into V[tok, dv] with a ones column appended.
  B) attention per (head, 512-token q-chunk): S^T = K-block^T-matmul(Q^T),
     causal tri mask on diag blocks, ACT exp -> P^T (bf16), then
     out[tq,129] += P^T-block.T @ [V|1] (rowsum rides in col 128),
     normalize, PE-transpose -> A^T [dv, t].
  C) y^T = wproj_s^T-blocks @ A^T, fp32 eviction, DMA out.
"""
import sys

if "/opt/trn_rl_repo" not in sys.path:
    sys.path.insert(0, "/opt/trn_rl_repo")

import math
import numpy as np
import ml_dtypes

B, T, N_EMBD = 2, 2048, 4096
HQ, HKV = 32, 8
HD = 128
N_CORES = 8
TPG = 4                      # tensor-parallel groups per batch
HQL, HKVL = HQ // TPG, HKV // TPG   # 8 q-heads, 2 kv-heads per core
SCALE = 1.0 / math.sqrt(HD)
BASE_FREQ = 10000.0
NEG = -1e30

bf16 = ml_dtypes.bfloat16


def build_nc(T=T, KE=N_EMBD, HQL=HQL, HKVL=HKVL, EOUT=N_EMBD, scale=SCALE):
    """Build the per-core Bass program. All shapes hardcoded at trace time."""
    import concourse.tile as tile
    from concourse import bacc, mybir

    f32 = mybir.dt.float32
    b16 = mybir.dt.bfloat16
    Exp = mybir.ActivationFunctionType.Exp
    mult = mybir.AluOpType.mult
    add = mybir.AluOpType.add

    KT = KE // 128          # contraction tiles for projections
    NKT = T // 128          # token tiles
    NCH = T // 512          # token chunks
    REP = HQL // HKVL

    nc = bacc.Bacc("TRN2", target_bir_lowering=False)

    xt_d = nc.dram_tensor("xt", [128, KT, T], b16, kind="ExternalInput")
    wq_d = nc.dram_tensor("wq", [128, HQL, KT, 128], b16, kind="ExternalInput")
    wk_d = nc.dram_tensor("wk", [128, HKVL, KT, 128], b16, kind="ExternalInput")
    wv_d = nc.dram_tensor("wv", [128, HKVL, KT, 128], b16, kind="ExternalInput")
    wp_d = nc.dram_tensor("wp", [128, HQL, EOUT], b16, kind="ExternalInput")
    cos_d = nc.dram_tensor("cos", [128, T], b16, kind="ExternalInput")
    rsin_d = nc.dram_tensor("rsin", [128, T], b16, kind="ExternalInput")
    tri_d = nc.dram_tensor("tri", [128, 128], f32, kind="ExternalInput")
    id_d = nc.dram_tensor("ident", [128, 128], b16, kind="ExternalInput")
    yt_d = nc.dram_tensor("yt", [EOUT, T], f32, kind="ExternalOutput")

    with tile.TileContext(nc) as tc:
        with tc.tile_pool(name="glob", bufs=1) as glob:
            cos_sb = glob.tile([128, T], b16)
            rsin_sb = glob.tile([128, T], b16)
            tri_sb = glob.tile([128, 128], f32)
            id_sb = glob.tile([128, 128], b16)
            nc.sync.dma_start(out=cos_sb[:], in_=cos_d[:])
            nc.sync.dma_start(out=rsin_sb[:], in_=rsin_d[:])
            nc.sync.dma_start(out=tri_sb[:], in_=tri_d[:])
            nc.sync.dma_start(out=id_sb[:], in_=id_d[:])

            qT = glob.tile([128, HQL, T], b16)       # rope(q)^T per head
            kT = glob.tile([128, HKVL, T], b16)      # rope(k)^T per head
            vON = glob.tile([128, HKVL, NKT, 129], b16)  # [tok, dv | 1]
            nc.vector.memset(vON[:, :, :, 128:129], 1.0)

            # ---------------- Phase A: projections -------------------------
            with tc.tile_pool(name="xt", bufs=1) as xtp, \
                 tc.tile_pool(name="wld", bufs=2) as wld, \
                 tc.tile_pool(name="rtmp", bufs=2) as rtmp, \
                 tc.tile_pool(name="vtmp", bufs=2) as vtmp, \
                 tc.tile_pool(name="psA", bufs=4, space="PSUM") as psA, \
                 tc.tile_pool(name="psR", bufs=2, space="PSUM") as psR, \
                 tc.tile_pool(name="psT", bufs=2, space="PSUM") as psT:

                # First weight slab before xt so PE can start as xt streams in.
                w_first = wld.tile([128, KT, 128], b16, tag="w", name="w_m")
                nc.sync.dma_start(out=w_first[:], in_=wq_d[:, 0])

                xt_sb = xtp.tile([128, KT, T], b16)
                for a in range(KT):
                    nc.sync.dma_start(out=xt_sb[:, a, :], in_=xt_d[:, a, :])

                def rope_evict(ps, dst, c):
                    # dst = ps * cos + shift64(ps) * rsin  (bf16 out)
                    cs = slice(512 * c, 512 * (c + 1))
                    t1 = rtmp.tile([128, 512], f32, tag="t1")
                    nc.vector.tensor_tensor(t1[0:64, :], ps[64:128, :],
                                            rsin_sb[0:64, cs], mult)
                    nc.vector.tensor_tensor(t1[64:128, :], ps[0:64, :],
                                            rsin_sb[64:128, cs], mult)
                    t2 = psR.tile([128, 512], f32, tag="t2")
                    nc.vector.tensor_tensor(t2[:], ps[:], cos_sb[:, cs], mult)
                    nc.vector.tensor_tensor(dst, t2[:], t1[:], add)

                # Q and K projections (transposed layout) with fused RoPE.
                # First two q-heads run k-outer over 4 live psums so PE can
                # chew each xt tile 4x as it lands from DRAM (startup ramp).
                for dst, w_d_, nh in ((qT, wq_d, HQL), (kT, wk_d, HKVL)):
                    for m in range(nh):
                        if dst is qT and m == 0:
                            w_m = w_first
                        else:
                            w_m = wld.tile([128, KT, 128], b16, tag="w")
                            nc.sync.dma_start(out=w_m[:], in_=w_d_[:, m])
                        if dst is qT and m < 2:
                            pss = [psA.tile([128, 512], f32, tag="pj",
                                            name=f"pj{c}") for c in range(NCH)]
                            for k in range(KT):
                                for c in range(NCH):
                                    nc.tensor.matmul(
                                        pss[c][:], lhsT=w_m[:, k, :],
                                        rhs=xt_sb[:, k, 512 * c:512 * (c + 1)],
                                        start=(k == 0), stop=(k == KT - 1))
                            for c in range(NCH):
                                rope_evict(pss[c],
                                           dst[:, m, 512 * c:512 * (c + 1)], c)
                            continue
                        for c in range(NCH):
                            ps = psA.tile([128, 512], f32, tag="pj")
                            for k in range(KT):
                                nc.tensor.matmul(
                                    ps[:], lhsT=w_m[:, k, :],
                                    rhs=xt_sb[:, k, 512 * c:512 * (c + 1)],
                                    start=(k == 0), stop=(k == KT - 1))
                            rope_evict(ps, dst[:, m, 512 * c:512 * (c + 1)], c)

                # V projection: v^T psum -> sbuf -> PE transpose -> vON
                for m in range(HKVL):
                    w_m = wld.tile([128, KT, 128], b16, tag="w")
                    nc.sync.dma_start(out=w_m[:], in_=wv_d[:, m])
                    for c in range(NCH):
                        ps = psA.tile([128, 512], f32, tag="pj")
                        for k in range(KT):
                            nc.tensor.matmul(
                                ps[:], lhsT=w_m[:, k, :],
                                rhs=xt_sb[:, k, 512 * c:512 * (c + 1)],
                                start=(k == 0), stop=(k == KT - 1))
                        vt = vtmp.tile([128, 512], b16, tag="vt")
                        nc.scalar.copy(out=vt[:], in_=ps[:])
                        for s in range(4):
                            pt = psT.tile([128, 128], b16, tag="tr")
                            nc.tensor.transpose(pt[:], vt[:, 128 * s:128 * (s + 1)],
                                                id_sb[:])
                            nc.scalar.copy(
                                out=vON[:, m, 4 * c + s, 0:128], in_=pt[:])

            # ---------------- Phases B + C ---------------------------------
            with tc.tile_pool(name="late", bufs=1) as late, \
                 tc.tile_pool(name="ppool", bufs=6) as ppool, \
                 tc.tile_pool(name="npool", bufs=4) as npool, \
                 tc.tile_pool(name="spool", bufs=4) as spool, \
                 tc.tile_pool(name="psS", bufs=3, space="PSUM") as psS, \
                 tc.tile_pool(name="psacc", bufs=1, space="PSUM") as psacc, \
                 tc.tile_pool(name="psT2", bufs=1, space="PSUM") as psT2:

                aT = late.tile([128, HQL, T], b16)
                wp_sb = late.tile([128, HQL, EOUT], b16)
                for k in range(HQL):
                    nc.sync.dma_start(out=wp_sb[:, k, :], in_=wp_d[:, k, :])

                # Phases B+C software-pipelined: while attention runs for
                # chunk c, the output projection for chunk c-1 is interleaved
                # between heads (4 e-tiles per head) so PE fills ACT-wait
                # gaps and the output DMA spreads across the whole run.
                def proj_tile(e, c):
                    ps = psS.tile([128, 512], f32, tag="s", name="psp")
                    for k in range(HQL):
                        nc.tensor.matmul(
                            ps[:], lhsT=wp_sb[:, k, 128 * e:128 * (e + 1)],
                            rhs=aT[:, k, 512 * c:512 * (c + 1)],
                            start=(k == 0), stop=(k == HQL - 1))
                    yt = ppool.tile([128, 512], f32, tag="yt", name="yt")
                    if e % 2 == 0:
                        nc.vector.tensor_copy(yt[:], ps[:])
                    else:
                        nc.scalar.copy(out=yt[:], in_=ps[:])
                    nc.sync.dma_start(
                        out=yt_d[128 * e:128 * (e + 1), 512 * c:512 * (c + 1)],
                        in_=yt[:])

                NE = EOUT // 128
                EPH = NE // HQL  # proj e-tiles interleaved per head
                for c in range(NCH):
                    for h in range(HQL):
                        v = h // REP
                        accs = [psacc.tile([128, 129], f32, tag=f"acc{s}",
                                           name=f"acc{s}")[:]
                                for s in range(4)]
                        n_tk = 4 * c + 4
                        pTs = {}

                        def vmms(t):
                            j = t - 4 * c
                            for s in range(4):
                                if j > s:
                                    continue
                                nc.tensor.matmul(
                                    accs[s],
                                    lhsT=pTs[t][:, 128 * s:128 * (s + 1)],
                                    rhs=vON[:, v, t, :],
                                    start=(t == 0), stop=(t == 4 * c + s))

                        # proj tiles of the previous chunk, interleaved into
                        # the t-loop so PE fills the ACT-paced gaps
                        pe_list = (list(range(EPH * h, EPH * (h + 1)))
                                   if c > 0 else [])

                        D = 3  # score->exp->V software-pipeline depth
                        for t in range(n_tk):
                            j = t - 4 * c  # >= 0 on diagonal-group tiles
                            col0 = 128 * j if j > 0 else 0
                            ps = psS.tile([128, 512], f32, tag="s")
                            nc.tensor.matmul(
                                ps[:, col0:512],
                                lhsT=kT[:, v, 128 * t:128 * (t + 1)],
                                rhs=qT[:, h, 512 * c + col0:512 * (c + 1)],
                                start=True, stop=True)
                            if j >= 0:
                                nc.vector.tensor_tensor(
                                    ps[:, 128 * j:128 * (j + 1)],
                                    ps[:, 128 * j:128 * (j + 1)],
                                    tri_sb[:], add)
                            pT = ppool.tile([128, 512], b16, tag="pT")
                            nc.scalar.activation(
                                pT[:, col0:512], ps[:, col0:512], Exp,
                                scale=scale)
                            pTs[t] = pT
                            if t >= D:
                                vmms(t - D)
                            if pe_list and \
                               (t + 1) * EPH // n_tk > t * EPH // n_tk:
                                proj_tile(pe_list.pop(0), c - 1)
                        for t in range(max(0, n_tk - D), n_tk):
                            vmms(t)
                        for e in pe_list:
                            proj_tile(e, c - 1)
                        for s in range(4):
                            rec = spool.tile([128, 1], f32, tag="rec")
                            nc.vector.reciprocal(rec[:], accs[s][:, 128:129])
                            an = npool.tile([128, 128], b16, tag="an")
                            nc.vector.tensor_scalar_mul(
                                an[:], accs[s][:, 0:128], rec[:])
                            pt = psT2.tile([128, 128], b16, tag="tr2")
                            nc.tensor.transpose(pt[:], an[:], id_sb[:])
                            cc = 512 * c + 128 * s
                            nc.vector.tensor_copy(aT[:, h, cc:cc + 128], pt[:])

                # drain: projection of the last chunk
                for e in range(NE):
                    proj_tile(e, NCH - 1)

    nc.compile()
    return nc


def _rope_tables(T=T):
    j = np.arange(64, dtype=np.float64)
    inv_freq = 1.0 / (BASE_FREQ ** (2.0 * j / HD))
    t = np.arange(T, dtype=np.float64)
    fr = t[:, None] * inv_freq[None, :]          # [T, 64]
    cos = np.cos(fr)                             # cos[t, d%64]
    sin = np.sin(fr)
    cos_tbl = np.concatenate([cos, cos], axis=1).T    # [128, T]
    rsin_tbl = np.concatenate([-sin, sin], axis=1).T  # [128, T]
    return cos_tbl.astype(bf16), rsin_tbl.astype(bf16)


def _pack_w(w):
    """[KE, M] -> [128, M//128, KE//128, 128]: w_l[p, m, a, j] = w[128a+p, 128m+j]."""
    KE, M = w.shape
    return np.ascontiguousarray(
        w.reshape(KE // 128, 128, M // 128, 128).transpose(1, 2, 0, 3))


def prep_core_inputs(x, wq, wk, wv, wproj):
    cos_tbl, rsin_tbl = _rope_tables()
    tri = np.where(np.arange(128)[None, :] >= np.arange(128)[:, None],
                   0.0, NEG).astype(np.float32)
    ident = np.eye(128, dtype=bf16)
    in_maps = []
    for ci in range(N_CORES):
        b, g = divmod(ci, TPG)
        xt = np.ascontiguousarray(
            x[b].T.reshape(N_EMBD // 128, 128, T).transpose(1, 0, 2)
        ).astype(bf16)
        qcols = slice(g * HQL * HD, (g + 1) * HQL * HD)
        kvcols = slice(g * HKVL * HD, (g + 1) * HKVL * HD)
        in_maps.append({
            "xt": xt,
            "wq": _pack_w(wq[:, qcols].astype(bf16)),
            "wk": _pack_w(wk[:, kvcols].astype(bf16)),
            "wv": _pack_w(wv[:, kvcols].astype(bf16)),
            "wp": np.ascontiguousarray(
                wproj[qcols, :].reshape(HQL, 128, N_EMBD).transpose(1, 0, 2)
            ).astype(bf16),
            "cos": cos_tbl, "rsin": rsin_tbl, "tri": tri, "ident": ident,
        })
    return in_maps


_NC_CACHE = {}


def _get_nc():
    if "nc" not in _NC_CACHE:
        _NC_CACHE["nc"] = build_nc()
    return _NC_CACHE["nc"]


def _get_runner():
    """Cached sharded-jit executor over the 8 cores (no donation, so the
    compiled executable is reusable across calls)."""
    if "runner" in _NC_CACHE:
        return _NC_CACHE["runner"]
    import jax
    from jax.sharding import Mesh, PartitionSpec, NamedSharding
    from jax.experimental.shard_map import shard_map
    from concourse import mybir
    from concourse.bass2jax import (_bass_exec_p, install_neuronx_cc_hook,
                                    partition_id_tensor)

    nc = _get_nc()
    install_neuronx_cc_hook()
    pname = nc.partition_id_tensor.name if nc.partition_id_tensor else None
    in_names, out_names, out_avals, zero_shapes = [], [], [], []
    for alloc in nc.m.functions[0].allocations:
        if not isinstance(alloc, mybir.MemoryLocationSet):
            continue
        name = alloc.memorylocations[0].name
        if alloc.kind == "ExternalInput":
            if name != pname:
                in_names.append(name)
        elif alloc.kind == "ExternalOutput":
            out_names.append(name)
            shape = tuple(alloc.tensor_shape)
            dtype = mybir.dt.np(alloc.dtype)
            out_avals.append(jax.core.ShapedArray(shape, dtype))
            zero_shapes.append((shape, dtype))
    all_names = in_names + out_names + ([pname] if pname else [])

    def _body(*args):
        operands = list(args)
        if pname:
            operands.append(partition_id_tensor())
        return tuple(_bass_exec_p.bind(
            *operands, out_avals=tuple(out_avals), in_names=tuple(all_names),
            out_names=tuple(out_names), lowering_input_output_aliases=(),
            sim_require_finite=True, sim_require_nnan=True, nc=nc))

    devices = jax.devices()[:N_CORES]
    mesh = Mesh(np.asarray(devices), ("core",))
    nin = len(in_names) + len(out_names)
    sharded = jax.jit(
        shard_map(_body, mesh=mesh, in_specs=(PartitionSpec("core"),) * nin,
                  out_specs=(PartitionSpec("core"),) * len(out_names),
                  check_rep=False),
        keep_unused=True)
    sh = NamedSharding(mesh, PartitionSpec("core"))
    zeros = [jax.device_put(
        np.zeros((N_CORES * s[0], *s[1:]), dt), sh)
        for s, dt in zero_shapes]

    def run(in_maps):
        concat = [np.concatenate([m[n] for m in in_maps], axis=0)
                  for n in in_names]
        dev_in = [jax.device_put(a, sh) for a in concat]
        outs = sharded(*dev_in, *zeros)
        jax.block_until_ready(outs)
        return [
            {n: np.asarray(outs[i]).reshape(N_CORES, *out_avals[i].shape)[ci]
             for i, n in enumerate(out_names)}
            for ci in range(N_CORES)]

    _NC_CACHE["runner"] = run
    return run


def kernel(x, wq, wk, wv, wproj):
    in_maps = prep_core_inputs(np.asarray(x, dtype=np.float32),
                               np.asarray(wq, dtype=np.float32),
                               np.asarray(wk, dtype=np.float32),
                               np.asarray(wv, dtype=np.float32),
                               np.asarray(wproj, dtype=np.float32))
    results = _get_runner()(in_maps)
    y = np.empty((B, T, N_EMBD), dtype=np.float32)
    for b in range(B):
        acc = results[b * TPG]["yt"].copy()
        for g in range(1, TPG):
            acc += results[b * TPG + g]["yt"]
        y[b] = acc.T
    return y


if __name__ == "__main__":
    rng = np.random.default_rng(0)
    x = rng.standard_normal((B, T, N_EMBD), dtype=np.float32)
    wq_ = (rng.standard_normal((N_EMBD, N_EMBD), dtype=np.float32) * 0.02)
    wk_ = (rng.standard_normal((N_EMBD, HKV * HD), dtype=np.float32) * 0.02)
    wv_ = (rng.standard_normal((N_EMBD, HKV * HD), dtype=np.float32) * 0.02)
    wp_ = (rng.standard_normal((N_EMBD, N_EMBD), dtype=np.float32) * 0.02)
    y = kernel(x, wq_, wk_, wv_, wp_)
    print("out", y.shape, y.dtype, float(np.abs(y).max()))
